# revision 2
# baseline (speedup 1.0000x reference)
"""Contrastive-loss kernel for trn2 (8 NeuronCores, SPMD).

The reference loss reduces to a Gram matrix G = F.T @ F over the
flattened input F [N=524288, T=64] (128 MiB fp32), followed by a tiny
[64,64] masked margin reduction.  Each core streams a contiguous
row-shard of F (16 MiB) through SBUF, casting fp32->bf16 inline in the
SWDGE DMA, and accumulates chunk.T @ chunk matmuls into one PSUM
accumulator (fp32).  The 8 partial [64,64] Grams are summed on the
host, where the masked margin reduction (negligible work) also runs.

v2 (from trace analysis of the v1 baseline):
  - The DMA stream runs at ~425 GB/s (per-NC HBM fair share) and is the
    bottleneck; NRT pre/postamble (~6.1us + ~7.8us incl. the 255-sem
    sweep, Tensor engine slowest at ~115ns/clear) is fixed overhead.
  - Tiles are mostly 2 MiB reads (fewer tile boundaries than v1's
    1 MiB) with a geometrically shrinking tail (down to 256 rows) so
    the PE lag after the last DMA byte is ~0.2us instead of ~2.2us.
  - All tiles get distinct SBUF slots (128 KiB/partition total), so
    there is no PE->DMA backpressure and each slot is written once;
    gpsimd emits every descriptor upfront and only waits for the
    output-store completion at the end.
  - Manual sem_clear/dma_reset teardown is dropped: NRT's postamble
    sweep resets all 255 user semaphores on every execution anyway
    (verified in the NTFF trace), so the manual pass was redundant
    serial tail work.

Semaphore protocol:
  - dma_sems[k]: SWDGE incs by 16 when tile k fully lands; PE waits 16.
  - pe_sem: last matmul incs 1 -> DVE combines PSUM diag blocks to SBUF.
  - out_sem: DVE incs -> sync engine stores the [64,64] result.
  - fin_sem: store-completion incs 16; gpsimd waits so no engine stream
    ends before the output has landed in HBM.
"""

import contextlib

import numpy as np

import concourse.bacc as bacc
import concourse.mybir as mybir
from concourse.bass_utils import run_bass_kernel_spmd

MARGIN = 60000.0
S = 64                      # time steps (Gram dim)
N_TOTAL = 2 * 8 * 32 * 32 * 32   # 524288 flattened rows
N_CORES = 8
N_SHARD = N_TOTAL // N_CORES     # 65536 rows per core
P = 128                     # SBUF partitions

# Tile sizes in rows: big 2 MiB-read tiles for stream efficiency, then a
# geometric tail so the last DMA completion gates almost no PE work.
TILE_ROWS = [8192] * 7 + [4096, 2048, 1024, 512, 256, 256]
assert sum(TILE_ROWS) == N_SHARD
N_TILES = len(TILE_ROWS)

_CACHE = {}
LAST_RESULTS = None         # BassKernelResults of the most recent run


def _build_nc():
    nc = bacc.Bacc("TRN2", target_bir_lowering=False, debug=False,
                   num_devices=N_CORES)
    x = nc.dram_tensor("x", [N_SHARD, S], mybir.dt.float32,
                       kind="ExternalInput")
    g = nc.dram_tensor("g", [S, S], mybir.dt.float32, kind="ExternalOutput")

    # Per-tile DRAM views: tile k covers rows [row0, row0+R); partition p
    # holds R/128 consecutive rows as one contiguous 256*(R/128)-byte
    # descriptor line.
    views = []
    row0 = 0
    for R in TILE_ROWS:
        views.append(x[row0:row0 + R, :].rearrange(
            "(p r) c -> p (r c)", p=P, r=R // P))
        row0 += R

    free_elems = [(R // P) * S for R in TILE_ROWS]   # bf16 elems/partition
    offs = [0]
    for fe in free_elems:
        offs.append(offs[-1] + fe)
    total_free = offs[-1]     # 65536 elems = 128 KiB/partition in bf16

    with (
        nc.sbuf_tensor("xbuf", [P, total_free], mybir.dt.bfloat16) as xbuf,
        nc.psum_tensor("acc", [2 * S, 2 * S], mybir.dt.float32) as acc,
        nc.sbuf_tensor("obuf", [S, S], mybir.dt.float32) as obuf,
        nc.semaphore("pe_sem") as pe_sem,
        nc.semaphore("out_sem") as out_sem,
        nc.semaphore("fin_sem") as fin_sem,
        contextlib.ExitStack() as stack,
    ):
        dma_sems = [stack.enter_context(nc.semaphore(f"dma_sem{k}"))
                    for k in range(N_TILES)]

        with nc.Block() as block:

            @block.gpsimd
            def _(gp):
                for k in range(N_TILES):
                    gp.dma_start(
                        xbuf[:, offs[k]:offs[k + 1]], views[k]
                    ).then_inc(dma_sems[k], 16)
                # keep this engine stream alive until the output store has
                # landed in HBM (NRT's postamble then resets all sems).
                gp.wait_ge(fin_sem, 16)

            @block.tensor
            def _(te):
                # Pack 2 row-chunks per matmul: lhsT = rhs = [A|B]
                # ([128, 128] bf16 -> FWL), accumulating
                # [[A'A, A'B], [B'A, B'B]] into a [128,128] PSUM tile.
                # The two diagonal 64x64 blocks sum to the Gram
                # contribution; off-diagonal blocks are discarded.
                total_mm = sum(fe // (2 * S) for fe in free_elems)
                n = 0
                for k in range(N_TILES):
                    te.wait_ge(dma_sems[k], 16)
                    for j in range(free_elems[k] // (2 * S)):
                        c = xbuf[:, offs[k] + j * 2 * S:
                                 offs[k] + (j + 1) * 2 * S]
                        mm = te.matmul(acc[:], c, c,
                                       start=(n == 0),
                                       stop=(n == total_mm - 1))
                        n += 1
                        if n == total_mm:
                            mm.then_inc(pe_sem, 1)

            @block.vector
            def _(v):
                v.wait_ge(pe_sem, 1)
                v.tensor_copy(obuf[:], acc[:S, :S])
                v.tensor_add(obuf[:], obuf[:],
                             acc[S:, S:]).then_inc(out_sem, 1)

            @block.sync
            def _(sy):
                sy.wait_ge(out_sem, 1)
                sy.dma_start(g[:], obuf[:]).then_inc(fin_sem, 16)

    nc.compile()
    return nc


def get_nc():
    if "nc" not in _CACHE:
        _CACHE["nc"] = _build_nc()
    return _CACHE["nc"]


def _device_partial_grams(flat: np.ndarray, **run_kwargs) -> np.ndarray:
    """Run the SPMD bass kernel; return the 8 partial Grams [8, 64, 64]."""
    global LAST_RESULTS
    nc = get_nc()
    in_maps = [
        {"x": flat[c * N_SHARD:(c + 1) * N_SHARD]} for c in range(N_CORES)
    ]
    LAST_RESULTS = run_bass_kernel_spmd(
        nc, in_maps, core_ids=list(range(N_CORES)), **run_kwargs
    )
    return np.stack([LAST_RESULTS.results[c]["g"] for c in range(N_CORES)])


def kernel(input: np.ndarray, **run_kwargs) -> np.ndarray:
    flat = np.ascontiguousarray(
        np.asarray(input, dtype=np.float32).reshape(N_TOTAL, S)
    )
    partials = _device_partial_grams(flat, **run_kwargs)

    gram = partials.astype(np.float64).sum(axis=0)
    sq = np.diag(gram)
    dist = sq[:, None] + sq[None, :] - 2.0 * gram
    idx = np.arange(S)
    lower = idx[:, None] > idx[None, :]
    adjacent = (idx[:, None] - idx[None, :]) == 1
    per_pair = np.where(adjacent, np.maximum(0.0, MARGIN - dist), dist)
    loss = np.where(lower, per_pair, 0.0).sum() / (S * (S - 1) * 1000)
    return np.asarray(loss, dtype=np.float32)


# revision 6
# speedup vs baseline: 1.0396x; 1.0396x over previous
"""Contrastive-loss kernel for trn2 (8 NeuronCores, SPMD).

The reference loss reduces to a Gram matrix G = F.T @ F over the
flattened input F [N=524288, T=64] (128 MiB fp32), followed by a tiny
[64,64] masked margin reduction.  Each core streams a contiguous
row-shard of F (16 MiB) through SBUF, casting fp32->bf16 inline in the
SWDGE DMA, and accumulates chunk.T @ chunk matmuls into one PSUM
accumulator (fp32).  The 8 partial [64,64] Grams are summed on the
host, where the masked margin reduction (negligible work) also runs.

v2 (from trace analysis of the v1 baseline):
  - The DMA stream runs at ~425 GB/s (per-NC HBM fair share) and is the
    bottleneck; NRT pre/postamble (~6.1us + ~7.8us incl. the 255-sem
    sweep, Tensor engine slowest at ~115ns/clear) is fixed overhead.
  - Tiles are mostly 2 MiB reads (fewer tile boundaries than v1's
    1 MiB) with a geometrically shrinking tail (down to 256 rows) so
    the PE lag after the last DMA byte is ~0.2us instead of ~2.2us.
  - All tiles get distinct SBUF slots (128 KiB/partition total), so
    there is no PE->DMA backpressure and each slot is written once;
    gpsimd emits every descriptor upfront and only waits for the
    output-store completion at the end.
  - Manual sem_clear/dma_reset teardown is dropped: NRT's postamble
    sweep resets all 255 user semaphores on every execution anyway
    (verified in the NTFF trace), so the manual pass was redundant
    serial tail work.

Semaphore protocol:
  - dma_sems[k]: SWDGE incs by 16 when tile k fully lands; PE waits 16.
  - pe_sem: last matmul incs 1 -> DVE combines PSUM diag blocks to SBUF.
  - out_sem: DVE incs -> sync engine stores the [64,64] result.
  - fin_sem: store-completion incs 16; gpsimd waits so no engine stream
    ends before the output has landed in HBM.
"""

import contextlib

import numpy as np

import concourse.bacc as bacc
import concourse.mybir as mybir
from concourse.bass_utils import run_bass_kernel_spmd

MARGIN = 60000.0
S = 64                      # time steps (Gram dim)
N_TOTAL = 2 * 8 * 32 * 32 * 32   # 524288 flattened rows
N_CORES = 8
N_SHARD = N_TOTAL // N_CORES     # 65536 rows per core
P = 128                     # SBUF partitions

# Tile sizes in rows: big 2 MiB-read tiles for stream efficiency, then a
# short tail so the last DMA completion gates little PE work.  (Many tiny
# tail tiles backfire: each extra DMA serializes a ~0.6us completion-
# receipt stall per engine at the end of the stream.)
TILE_ROWS = [8192] * 7 + [4096, 2048, 2048]
assert sum(TILE_ROWS) == N_SHARD
N_TILES = len(TILE_ROWS)

_CACHE = {}
LAST_RESULTS = None         # BassKernelResults of the most recent run


def _build_nc():
    nc = bacc.Bacc("TRN2", target_bir_lowering=False, debug=False,
                   num_devices=N_CORES)
    x = nc.dram_tensor("x", [N_SHARD, S], mybir.dt.float32,
                       kind="ExternalInput")
    g = nc.dram_tensor("g", [S, S], mybir.dt.float32, kind="ExternalOutput")

    # Per-tile DRAM views: tile k covers rows [row0, row0+R); partition p
    # holds R/128 consecutive rows as one contiguous 256*(R/128)-byte
    # descriptor line.
    views = []
    row0 = 0
    for R in TILE_ROWS:
        views.append(x[row0:row0 + R, :].rearrange(
            "(p r) c -> p (r c)", p=P, r=R // P))
        row0 += R

    free_elems = [(R // P) * S for R in TILE_ROWS]   # bf16 elems/partition
    offs = [0]
    for fe in free_elems:
        offs.append(offs[-1] + fe)
    total_free = offs[-1]     # 65536 elems = 128 KiB/partition in bf16

    with (
        nc.sbuf_tensor("xbuf", [P, total_free], mybir.dt.bfloat16) as xbuf,
        nc.psum_tensor("acc", [2 * S, 2 * S], mybir.dt.float32) as acc,
        nc.sbuf_tensor("obuf", [S, S], mybir.dt.float32) as obuf,
        nc.semaphore("pe_sem") as pe_sem,
        nc.semaphore("out_sem") as out_sem,
        nc.semaphore("fin_sem") as fin_sem,
        contextlib.ExitStack() as stack,
    ):
        dma_sems = [stack.enter_context(nc.semaphore(f"dma_sem{k}"))
                    for k in range(N_TILES)]

        with nc.Block() as block:

            @block.gpsimd
            def _(gp):
                for k in range(N_TILES):
                    gp.dma_start(
                        xbuf[:, offs[k]:offs[k + 1]], views[k]
                    ).then_inc(dma_sems[k], 16)
                # keep this engine stream alive until the output store has
                # landed in HBM (NRT's postamble then resets all sems).
                gp.wait_ge(fin_sem, 16)
                # narrow ring-state reset here keeps the bacc block-end
                # DRAIN short (~50ns instead of ~1.3us of quiesce work).
                gp.dma_reset()

            @block.tensor
            def _(te):
                # Pack 2 row-chunks per matmul: lhsT = rhs = [A|B]
                # ([128, 128] bf16 -> FWL), accumulating
                # [[A'A, A'B], [B'A, B'B]] into a [128,128] PSUM tile.
                # The two diagonal 64x64 blocks sum to the Gram
                # contribution; off-diagonal blocks are discarded.
                total_mm = sum(fe // (2 * S) for fe in free_elems)
                n = 0
                for k in range(N_TILES):
                    te.wait_ge(dma_sems[k], 16)
                    for j in range(free_elems[k] // (2 * S)):
                        c = xbuf[:, offs[k] + j * 2 * S:
                                 offs[k] + (j + 1) * 2 * S]
                        mm = te.matmul(acc[:], c, c,
                                       start=(n == 0),
                                       stop=(n == total_mm - 1))
                        n += 1
                        if n == total_mm:
                            mm.then_inc(pe_sem, 1)

            @block.vector
            def _(v):
                v.wait_ge(pe_sem, 1)
                v.tensor_copy(obuf[:], acc[:S, :S])
                v.tensor_add(obuf[:], obuf[:],
                             acc[S:, S:]).then_inc(out_sem, 1)

            @block.sync
            def _(sy):
                sy.wait_ge(out_sem, 1)
                sy.dma_start(g[:], obuf[:]).then_inc(fin_sem, 16)

    nc.compile()
    return nc


def get_nc():
    if "nc" not in _CACHE:
        _CACHE["nc"] = _build_nc()
    return _CACHE["nc"]


def _device_partial_grams(flat: np.ndarray, **run_kwargs) -> np.ndarray:
    """Run the SPMD bass kernel; return the 8 partial Grams [8, 64, 64]."""
    global LAST_RESULTS
    nc = get_nc()
    in_maps = [
        {"x": flat[c * N_SHARD:(c + 1) * N_SHARD]} for c in range(N_CORES)
    ]
    LAST_RESULTS = run_bass_kernel_spmd(
        nc, in_maps, core_ids=list(range(N_CORES)), **run_kwargs
    )
    return np.stack([LAST_RESULTS.results[c]["g"] for c in range(N_CORES)])


def kernel(input: np.ndarray, **run_kwargs) -> np.ndarray:
    flat = np.ascontiguousarray(
        np.asarray(input, dtype=np.float32).reshape(N_TOTAL, S)
    )
    partials = _device_partial_grams(flat, **run_kwargs)

    gram = partials.astype(np.float64).sum(axis=0)
    sq = np.diag(gram)
    dist = sq[:, None] + sq[None, :] - 2.0 * gram
    idx = np.arange(S)
    lower = idx[:, None] > idx[None, :]
    adjacent = (idx[:, None] - idx[None, :]) == 1
    per_pair = np.where(adjacent, np.maximum(0.0, MARGIN - dist), dist)
    loss = np.where(lower, per_pair, 0.0).sum() / (S * (S - 1) * 1000)
    return np.asarray(loss, dtype=np.float32)
